# revision 1
# baseline (speedup 1.0000x reference)
"""SSD DetectionLoss Trainium2 kernel — data-parallel over batch across 8 NeuronCores.

Self-contained: hardcodes shapes from the problem spec (B=32, N=32768, C=81, M=40).
Host side only slices/replicates/relayouts inputs, runs the Bass kernel SPMD on 8
cores, and does the final 8-way scalar reduction + normalization.

Per-core algorithm (4 images):
  - anchors live one-per-partition-column: anchor n = p*256 + a  (p partition, a column)
  - per anchor-column chunk, IoU vs all (img, gt) pairs is built with fused
    scalar_tensor_tensor ops; max/argmax via reduce + is_ge-against-max(thr,0.5)
    indicator (pos-masked).
  - target-class logit sums via PE matmul  G[m,c] += ind[p,m]^T @ logits[p,c]
  - matched gt coords via PE transpose of ind + fp16 gather matmul vs gt table
  - CE: exp on ACT, per-anchor sum on DVE, ln on ACT;  ce0 = lse - logit[:,0]
  - hard-negative mining: two-level grid over t of  E(t)=sum(relu(ce_neg-t));
    neg_sum = min_t E(t) + k*t   (exact up to tiny convexity gap)
"""

import numpy as np

import concourse.bass as bass
import concourse.bacc as bacc
import concourse.mybir as mybir
import concourse.tile as tile
from concourse.bass_utils import run_bass_kernel_spmd

F32 = mybir.dt.float32
F16 = mybir.dt.float16
U32 = mybir.dt.uint32
AX = mybir.AxisListType
OP = mybir.AluOpType
ACT = mybir.ActivationFunctionType

# problem constants
B, N, C, M = 32, 32768, 81, 40
NCORES = 8
BPC = B // NCORES          # images per core
P = 128                    # partitions
NEG_FB = float(int(N * 0.05))
EPS = 1e-10

# mining grid
J = 32
TLO1, DT1 = 0.25, 0.375
DT2 = 2.0 * DT1 / J


def build_nc(bpc=BPC, n=N, nclass=C, m=M, a_sub=16):
    """Build the single-core Bass/Tile program (same program for all 8 cores)."""
    nc = bacc.Bacc(None)
    na = n // P               # anchor columns per partition
    ngrp = na // a_sub
    half = bpc * m // 2       # rows per transposed ind half

    # ---------------- DRAM I/O ----------------
    logits_d = nc.dram_tensor("logits", [bpc, n, nclass], F32, kind="ExternalInput")
    pred_d = nc.dram_tensor("pred", [bpc, n, 4], F32, kind="ExternalInput")
    db_d = nc.dram_tensor("db", [n, 4], F32, kind="ExternalInput")
    gtrep_d = nc.dram_tensor("gtrep", [P, 4, bpc, m], F32, kind="ExternalInput")
    gt40_d = nc.dram_tensor("gt40", [m, bpc, 5], F32, kind="ExternalInput")
    nhalves = (bpc + max(bpc // 2, 1) - 1) // max(bpc // 2, 1)
    gt40h_d = nc.dram_tensor("gt40h", [P, nhalves, max(bpc // 2, 1) * 4], F16,
                             kind="ExternalInput")
    ident_d = nc.dram_tensor("ident", [P, P], F32, kind="ExternalInput")
    iota32_d = nc.dram_tensor("iota32", [1, J], F32, kind="ExternalInput")
    iota81_d = nc.dram_tensor("iota81", [m, nclass], F32, kind="ExternalInput")
    out_d = nc.dram_tensor("out", [1, 32], F32, kind="ExternalOutput")

    with tile.TileContext(nc) as tc:
        with (
            tc.tile_pool(name="per", bufs=1) as per,           # persistent SBUF
            tc.tile_pool(name="lgtp", bufs=2) as lgtp,         # logits group tiles
            tc.tile_pool(name="expp", bufs=3) as expp,
            tc.tile_pool(name="pair", bufs=3) as pairp,        # per-chunk pair tiles
            tc.tile_pool(name="smal", bufs=1) as smal,         # small transient
            tc.tile_pool(name="ps_g", bufs=1, space="PSUM") as ps_g,
            tc.tile_pool(name="ps_tr", bufs=2, space="PSUM") as ps_tr,
            tc.tile_pool(name="ps_mt", bufs=1, space="PSUM") as ps_mt,
            tc.tile_pool(name="ps_ms", bufs=1, space="PSUM") as ps_ms,
        ):
            # ---------------- prep ----------------
            dbt = per.tile([P, na * 4], F32)
            nc.sync.dma_start(dbt[:], db_d[:].rearrange("(p a) j -> p (a j)", p=P))
            dbv = dbt[:].rearrange("p (a j) -> p j a", j=4)  # strided views
            dbx1, dby1, dbx2, dby2 = dbv[:, 0, :], dbv[:, 1, :], dbv[:, 2, :], dbv[:, 3, :]

            areab = per.tile([P, na], F32)
            tmpa = smal.tile([P, na], F32, tag="tmpa")
            tmpb = smal.tile([P, na], F32, tag="tmpb")
            nc.vector.tensor_tensor(tmpa[:], dbx2, dbx1, OP.subtract)
            nc.vector.tensor_tensor(tmpb[:], dby2, dby1, OP.subtract)
            nc.vector.tensor_tensor(areab[:], tmpa[:], tmpb[:], OP.mult)

            gtt = per.tile([P, 4, bpc, m], F32)
            nc.sync.dma_start(gtt[:], gtrep_d[:])
            gx1 = gtt[:, 0].rearrange("p b m -> p (b m)")
            gy1 = gtt[:, 1].rearrange("p b m -> p (b m)")
            gx2 = gtt[:, 2].rearrange("p b m -> p (b m)")
            gy2 = gtt[:, 3].rearrange("p b m -> p (b m)")

            garea = per.tile([P, bpc * m], F32)
            tmpc = smal.tile([P, bpc * m], F32, tag="tmpc")
            tmpd = smal.tile([P, bpc * m], F32, tag="tmpd")
            nc.vector.tensor_tensor(tmpc[:], gx2, gx1, OP.subtract)
            nc.vector.tensor_tensor(tmpd[:], gy2, gy1, OP.subtract)
            # garea = gw*gh + EPS
            nc.vector.scalar_tensor_tensor(garea[:], tmpc[:], 1.0, tmpd[:], OP.mult, OP.mult)
            nc.vector.tensor_scalar(garea[:], garea[:], EPS, None, OP.add)

            gt40t = per.tile([m, bpc, 5], F32)
            nc.sync.dma_start(gt40t[:], gt40_d[:])
            gt40h = per.tile([P, nhalves, max(bpc // 2, 1) * 4], F16)
            nc.sync.dma_start(gt40h[:], gt40h_d[:])
            identt = per.tile([P, P], F32)
            nc.sync.dma_start(identt[:], ident_d[:])
            iota32t = per.tile([1, J], F32)
            nc.sync.dma_start(iota32t[:], iota32_d[:])
            iota81t = per.tile([m, nclass], F32)
            nc.sync.dma_start(iota81t[:], iota81_d[:])

            ones_col = per.tile([P, 1], F32)
            nc.gpsimd.memset(ones_col[:], 1.0)
            ones_row1 = per.tile([1, P], F32)
            nc.gpsimd.memset(ones_row1[:], 1.0)
            ones40 = per.tile([m, 1], F32)
            nc.gpsimd.memset(ones40[:], 1.0)
            zerot = per.tile([P, na], F32)
            nc.gpsimd.memset(zerot[:], 0.0)

            # pred -> xyxy
            px = []
            for i in range(bpc):
                prt = smal.tile([P, na * 4], F32, tag="prt", bufs=2)
                nc.sync.dma_start(prt[:], pred_d[i].rearrange("(p a) j -> p (a j)", p=P))
                prv = prt[:].rearrange("p (a j) -> p j a", j=4)
                pxi = per.tile([P, 4, na], F32, name=f"px{i}")
                nc.vector.scalar_tensor_tensor(pxi[:, 0], prv[:, 2, :], -0.5, prv[:, 0, :], OP.mult, OP.add)
                nc.vector.scalar_tensor_tensor(pxi[:, 1], prv[:, 3, :], -0.5, prv[:, 1, :], OP.mult, OP.add)
                nc.vector.scalar_tensor_tensor(pxi[:, 2], prv[:, 2, :], 0.5, prv[:, 0, :], OP.mult, OP.add)
                nc.vector.scalar_tensor_tensor(pxi[:, 3], prv[:, 3, :], 0.5, prv[:, 1, :], OP.mult, OP.add)
                px.append(pxi)

            # persistent wide result tiles
            maxiou = per.tile([P, bpc, na], F32)
            lsum = per.tile([P, bpc, na], F32)     # sum(exp) per anchor
            x0t = per.tile([P, bpc, na], F32)
            matched = per.tile([P, na, bpc * 4], F32)   # matched gt coords
            g_ps = [ps_g.tile([m, nclass], F32, name=f"gps{i}", tag=f"gps{i}")
                    for i in range(bpc)]

            # ---------------- main loop ----------------
            lgr = logits_d[:].rearrange("b (p a) c -> b p a c", p=P)
            trt = None
            trs = None
            for g in range(ngrp):
                lgt_tiles = []
                for i in range(bpc):
                    lgt = lgtp.tile([P, a_sub, nclass], F32, tag=f"lgt{i}")
                    nc.sync.dma_start(lgt[:], lgr[i, :, g * a_sub:(g + 1) * a_sub, :])
                    lgt_tiles.append(lgt)
                    ext = expp.tile([P, a_sub, nclass], F32, tag="ext")
                    nc.scalar.activation(ext[:], lgt[:], ACT.Exp)
                    nc.vector.tensor_reduce(
                        lsum[:, i, g * a_sub:(g + 1) * a_sub], ext[:], AX.X, OP.add)
                    nc.vector.tensor_copy(x0t[:, i, g * a_sub:(g + 1) * a_sub], lgt[:, :, 0])

                for al in range(a_sub):
                    a = g * a_sub + al
                    t2 = pairp.tile([P, bpc * m], F32, tag="t2")
                    w = pairp.tile([P, bpc * m], F32, tag="w")
                    h = pairp.tile([P, bpc * m], F32, tag="h")
                    inter = pairp.tile([P, bpc * m], F32, tag="inter")
                    den = pairp.tile([P, bpc * m], F32, tag="den")
                    rec = pairp.tile([P, bpc * m], F32, tag="rec")
                    iou = pairp.tile([P, bpc, m], F32, tag="iou")
                    mpad = 64  # gt dim padded so transposed img rows land at 0/64
                    ind = pairp.tile([P, bpc, mpad], F32, tag="ind")
                    nc.gpsimd.memset(ind[:, :, m:mpad], 0.0)
                    iouf = iou[:].rearrange("p b m -> p (b m)")

                    nc.vector.tensor_scalar(t2[:], gx1, dbx1[:, a:a + 1], None, OP.max)
                    nc.vector.scalar_tensor_tensor(w[:], gx2, dbx2[:, a:a + 1], t2[:], OP.min, OP.subtract)
                    nc.vector.tensor_scalar(t2[:], gy1, dby1[:, a:a + 1], None, OP.max)
                    nc.vector.scalar_tensor_tensor(h[:], gy2, dby2[:, a:a + 1], t2[:], OP.min, OP.subtract)
                    nc.vector.scalar_tensor_tensor(inter[:], w[:], 0.0, h[:], OP.max, OP.mult)
                    nc.vector.scalar_tensor_tensor(den[:], garea[:], areab[:, a:a + 1], inter[:], OP.add, OP.subtract)
                    nc.vector.reciprocal_approx_fast(rec[:], den[:])
                    nc.gpsimd.tensor_tensor(iouf, inter[:], rec[:], OP.mult)
                    nc.vector.tensor_reduce(maxiou[:, :, a], iou[:], AX.X, OP.max)
                    thr4 = smal.tile([P, bpc], F32, tag="thr4", bufs=3)
                    nc.vector.tensor_scalar(thr4[:], maxiou[:, :, a], 0.5, None, OP.max)
                    for i in range(bpc):
                        nc.vector.tensor_scalar(ind[:, i, 0:m], iou[:, i], thr4[:, i:i + 1], None, OP.is_ge)

                    # G matmuls (accumulate target-class logit sums)
                    for i in range(bpc):
                        nc.tensor.matmul(
                            g_ps[i][:], ind[:, i, 0:m], lgt_tiles[i][:, al, :],
                            start=(a == 0), stop=(a == na - 1))

                    # transpose ind halves into PSUM; 2 chunks per [P, 4P] tile
                    nh = max(bpc // 2, 1)  # imgs per transpose half
                    if a % 2 == 0:
                        trt = ps_tr.tile([nh * mpad, 4 * P], F32, tag="trt")
                        trs = smal.tile([nh * mpad, 4 * P], F16, tag="trs", bufs=2)
                    co = (a % 2) * 2 * P
                    nc.tensor.transpose(
                        trt[:, co:co + P],
                        ind[:, 0:nh].rearrange("p b m -> p (b m)"), identt[:])
                    if bpc > 1:
                        nc.tensor.transpose(
                            trt[:, co + P:co + 2 * P],
                            ind[:, nh:].rearrange("p b m -> p (b m)"), identt[:])
                    if a % 2 == 1:
                        nc.scalar.activation(trs[:], trt[:], ACT.Copy)
                        nhalves = (bpc + nh - 1) // nh
                        for (aa, col0) in ((a - 1, 0), (a, 2 * P)):
                            mt = ps_mt.tile([P, bpc * 4], F32, tag="mt")
                            for hh in range(nhalves):
                                # block-diagonal gt table: img ii of this half
                                # occupies rows ii*mpad..ii*mpad+m, cols ii*4..
                                nc.tensor.matmul(
                                    mt[:, hh * nh * 4:(hh + 1) * nh * 4],
                                    trs[0:nh * mpad, col0 + hh * P:col0 + (hh + 1) * P],
                                    gt40h[0:nh * mpad, hh, 0:nh * 4],
                                    start=True, stop=True)
                            nc.vector.tensor_copy(matched[:, aa, :], mt[:])

            # ---------------- post: ce, masks, counts ----------------
            lse = per.tile([P, bpc, na], F32)
            nc.scalar.activation(lse[:].rearrange("p b a -> p (b a)"),
                                 lsum[:].rearrange("p b a -> p (b a)"), ACT.Ln)
            ce0 = per.tile([P, bpc, na], F32)
            nc.vector.tensor_tensor(ce0[:].rearrange("p b a -> p (b a)"),
                                    lse[:].rearrange("p b a -> p (b a)"),
                                    x0t[:].rearrange("p b a -> p (b a)"), OP.subtract)
            pos01 = per.tile([P, bpc, na], F32)
            nc.vector.tensor_scalar(pos01[:].rearrange("p b a -> p (b a)"),
                                    maxiou[:].rearrange("p b a -> p (b a)"), 0.5, None, OP.is_ge)
            neg01 = per.tile([P, bpc, na], F32)
            nc.vector.tensor_scalar(neg01[:].rearrange("p b a -> p (b a)"),
                                    maxiou[:].rearrange("p b a -> p (b a)"), 0.4, None, OP.is_lt)
            ce_neg = per.tile([P, bpc, na], F32)
            nc.vector.tensor_tensor(ce_neg[:].rearrange("p b a -> p (b a)"),
                                    ce0[:].rearrange("p b a -> p (b a)"),
                                    neg01[:].rearrange("p b a -> p (b a)"), OP.mult)

            pack = per.tile([P, 16], F32)
            nc.vector.memset(pack[:], 0.0)
            nc.vector.tensor_reduce(pack[:, 0:bpc], pos01[:], AX.X, OP.add)
            nc.vector.tensor_reduce(pack[:, 4:4 + bpc], neg01[:], AX.X, OP.add)
            plse = smal.tile([P, bpc, na], F32, tag="plse")
            nc.vector.tensor_tensor(plse[:].rearrange("p b a -> p (b a)"),
                                    lse[:].rearrange("p b a -> p (b a)"),
                                    pos01[:].rearrange("p b a -> p (b a)"), OP.mult)
            nc.vector.tensor_reduce(pack[:, 8:8 + bpc], plse[:], AX.X, OP.add)

            # ---------------- loc ----------------
            slacc = smal.tile([P, na], F32, tag="slacc")
            dtl = smal.tile([P, na], F32, tag="dtl")
            atl = smal.tile([P, na], F32, tag="atl")
            mtl = smal.tile([P, na], F32, tag="mtl")
            ttl = smal.tile([P, na], F32, tag="ttl")
            sltl = smal.tile([P, na], F32, tag="sltl")
            mview = matched[:].rearrange("p a (b j) -> p b j a", b=bpc)
            for i in range(bpc):
                for j in range(4):
                    nc.vector.tensor_tensor(dtl[:], px[i][:, j], mview[:, i, j, :], OP.subtract)
                    nc.vector.scalar_tensor_tensor(atl[:], dtl[:], -1.0, dtl[:], OP.mult, OP.max)
                    nc.vector.tensor_scalar(mtl[:], atl[:], 1.0, None, OP.min)
                    nc.vector.scalar_tensor_tensor(ttl[:], mtl[:], -0.5, atl[:], OP.mult, OP.add)
                    if j == 0:
                        nc.vector.tensor_tensor(slacc[:], mtl[:], ttl[:], OP.mult)
                    else:
                        nc.vector.tensor_tensor(sltl[:], mtl[:], ttl[:], OP.mult)
                        nc.vector.tensor_tensor(slacc[:], slacc[:], sltl[:], OP.add)
                nc.vector.tensor_tensor(slacc[:], slacc[:], pos01[:, i], OP.mult)
                nc.vector.tensor_reduce(pack[:, 12 + i:13 + i], slacc[:], AX.X, OP.add)

            # partition reduce of pack via PE
            s16p = ps_ms.tile([1, 16], F32, tag="ms")
            nc.tensor.matmul(s16p[:], ones_col[:], pack[:], start=True, stop=True)
            s16 = per.tile([1, 16], F32)
            nc.vector.tensor_copy(s16[:], s16p[:])

            # ---------------- P_corr ----------------
            pcp = ps_ms.tile([1, 1], F32, tag="ms")
            ohx = smal.tile([m, nclass], F32, tag="ohx")
            gsel = smal.tile([m, nclass], F32, tag="gsel")
            gpart = smal.tile([m, 1], F32, tag="gpart")
            lab1 = smal.tile([m, 1], F32, tag="lab1")
            for i in range(bpc):
                nc.vector.tensor_scalar(lab1[:], gt40t[:, i, 4:5], 1.0, None, OP.add)
                nc.vector.tensor_scalar(ohx[:], iota81t[:], lab1[:], None, OP.is_equal)
                nc.vector.tensor_tensor(gsel[:], g_ps[i][:], ohx[:], OP.mult)
                nc.vector.tensor_reduce(gpart[:], gsel[:], AX.X, OP.add)
                nc.tensor.matmul(pcp[:], gpart[:], ones40[:], start=(i == 0), stop=(i == bpc - 1))
            pcs = smal.tile([1, 1], F32, tag="pcs")
            nc.vector.tensor_copy(pcs[:], pcp[:])

            # ---------------- mining ----------------
            cen = ce_neg[:].rearrange("p b a -> p b a")
            ep1 = per.tile([P, J, bpc], F32)
            for jj in range(J):
                tj = TLO1 + DT1 * jj
                for i in range(bpc):
                    nc.vector.scalar_tensor_tensor(
                        dtl[:], cen[:, i], float(tj), zerot[:], OP.subtract, OP.max,
                        accum_out=ep1[:, jj, i:i + 1])
            e1p = ps_ms.tile([1, J * bpc], F32, tag="ms")
            nc.tensor.matmul(e1p[:], ones_col[:], ep1[:].rearrange("p j b -> p (j b)"),
                             start=True, stop=True)
            e1 = per.tile([1, J * bpc], F32)
            nc.vector.tensor_copy(e1[:], e1p[:])
            e1v = e1[:].rearrange("o (j b) -> o j b", b=bpc)

            # k per image (exact small-integer arithmetic in f32)
            kt = per.tile([1, bpc], F32)
            k3 = smal.tile([1, bpc], F32, tag="k3")
            kf = smal.tile([1, bpc], F32, tag="kf")
            ks = smal.tile([1, bpc], F32, tag="ks")
            npr = s16[:, 0:bpc]
            nnr = s16[:, 4:4 + bpc]
            nc.vector.tensor_scalar(k3[:], npr, 3.0, None, OP.mult)
            nc.vector.tensor_tensor(k3[:], k3[:], nnr, OP.min)
            nc.vector.tensor_scalar(kf[:], nnr, NEG_FB, None, OP.min)
            nc.vector.tensor_scalar(ks[:], npr, 0.0, None, OP.is_gt)
            nc.vector.tensor_tensor(k3[:], k3[:], ks[:], OP.mult)
            nc.vector.tensor_scalar(ks[:], ks[:], -1.0, 1.0, OP.mult, OP.add)  # 1 - s
            nc.vector.tensor_tensor(kf[:], kf[:], ks[:], OP.mult)
            nc.vector.tensor_tensor(kt[:], k3[:], kf[:], OP.add)

            negsum = per.tile([1, 1], F32)
            nsacc = smal.tile([1, 1], F32, tag="nsacc")
            s1t = smal.tile([1, J], F32, tag="s1t")
            n1t = smal.tile([1, J], F32, tag="n1t")
            m8 = smal.tile([1, 8], F32, tag="m8")
            i8 = smal.tile([1, 8], U32, tag="i8")
            idxf = smal.tile([1, 1], F32, tag="idxf")
            tstar = smal.tile([1, 1], F32, tag="tstar")
            kdt = smal.tile([1, 1], F32, tag="kdt")
            t2r = smal.tile([1, J], F32, tag="t2r")
            ep2 = per.tile([P, J, bpc], F32)
            t2s = per.tile([P, J], F32)
            s2t = smal.tile([1, J], F32, tag="s2t")
            for i in range(bpc):
                # S1 = E1 + k*(TLO1 + DT1*j)
                nc.vector.tensor_scalar(kdt[:], kt[:, i:i + 1], DT1, None, OP.mult)
                nc.vector.scalar_tensor_tensor(s1t[:], iota32t[:], kdt[:], e1v[:, :, i], OP.mult, OP.add)
                nc.vector.tensor_scalar(kdt[:], kt[:, i:i + 1], TLO1, None, OP.mult)
                nc.vector.tensor_scalar(s1t[:], s1t[:], kdt[:], None, OP.add)
                nc.vector.tensor_scalar(n1t[:], s1t[:], -1.0, None, OP.mult)
                nc.vector.max(m8[:], n1t[:])
                nc.vector.max_index(i8[:], m8[:], n1t[:])
                nc.vector.tensor_copy(idxf[:], i8[:, 0:1])
                # t2base = max(TLO1 + DT1*idx - DT1, 1e-3)
                nc.vector.tensor_scalar(tstar[:], idxf[:], DT1, TLO1 - DT1, OP.mult, OP.add)
                nc.vector.tensor_scalar(tstar[:], tstar[:], 1e-3, None, OP.max)
                nc.vector.tensor_scalar(t2r[:], iota32t[:], DT2, tstar[:], OP.mult, OP.add)
                t2b = ps_ms.tile([P, J], F32, tag="ms")
                nc.tensor.matmul(t2b[:], ones_row1[:], t2r[:], start=True, stop=True)
                nc.vector.tensor_copy(t2s[:], t2b[:])
                for jj in range(J):
                    nc.vector.scalar_tensor_tensor(
                        dtl[:], cen[:, i], t2s[:, jj:jj + 1], zerot[:], OP.subtract, OP.max,
                        accum_out=ep2[:, jj, i:i + 1])
                e2p = ps_ms.tile([1, J], F32, tag="ms")
                nc.tensor.matmul(e2p[:], ones_col[:], ep2[:, :, i], start=True, stop=True)
                nc.vector.tensor_copy(s2t[:], e2p[:])
                nc.vector.scalar_tensor_tensor(s2t[:], t2r[:], kt[:, i:i + 1], s2t[:], OP.mult, OP.add)
                nc.vector.tensor_reduce(nsacc[:], s2t[:], AX.X, OP.min)
                if i == 0:
                    nc.vector.tensor_copy(negsum[:], nsacc[:])
                else:
                    nc.vector.tensor_tensor(negsum[:], negsum[:], nsacc[:], OP.add)

            # ---------------- assemble output ----------------
            outt = per.tile([1, 32], F32)
            nc.vector.memset(outt[:], 0.0)
            acc1 = smal.tile([1, 1], F32, tag="acc1")
            for base, slot in ((12, 0), (8, 1), (0, 4), (4, 5)):
                nc.vector.tensor_reduce(acc1[:], s16[:, base:base + bpc], AX.X, OP.add)
                nc.vector.tensor_copy(outt[:, slot:slot + 1], acc1[:])
            nc.vector.tensor_copy(outt[:, 2:3], pcs[:])
            nc.vector.tensor_copy(outt[:, 3:4], negsum[:])
            nc.vector.tensor_copy(outt[:, 8:8 + bpc], s16[:, 0:bpc])
            nc.vector.tensor_copy(outt[:, 12:12 + bpc], kt[:])
            nc.sync.dma_start(out_d[:], outt[:])

    nc.compile()
    return nc


_NC_CACHE = {}


def _get_nc():
    if "nc" not in _NC_CACHE:
        _NC_CACHE["nc"] = build_nc()
    return _NC_CACHE["nc"]


def host_prep(cls_logits, bbox_pred_cxcywh, gt_boxes, gt_labels, default_boxes_xyxy,
              ncores=NCORES, bpc=BPC, m=M, nclass=C):
    """Slice/replicate/relayout inputs per core. No arithmetic on tensor data."""
    in_maps = []
    ident = np.eye(P, dtype=np.float32)
    iota32 = np.arange(J, dtype=np.float32).reshape(1, J)
    iota81 = np.broadcast_to(np.arange(nclass, dtype=np.float32), (m, nclass)).copy()
    for c in range(ncores):
        s = slice(c * bpc, (c + 1) * bpc)
        gtb = gt_boxes[s]                        # [bpc, M, 4]
        gtl = gt_labels[s].astype(np.float32)    # [bpc, M]
        gtrep = np.ascontiguousarray(
            np.broadcast_to(gtb.transpose(2, 0, 1)[None], (P, 4, bpc, m)))
        gt40 = np.ascontiguousarray(
            np.concatenate([gtb, gtl[:, :, None]], axis=2).transpose(1, 0, 2))
        # block-diagonal fp16 gt table: per transpose-half hh, image ii-of-half
        # occupies rows ii*64..ii*64+M and cols ii*4..(ii+1)*4; zeros elsewhere.
        nh = max(bpc // 2, 1)
        nhalves = (bpc + nh - 1) // nh
        gt40h = np.zeros((P, nhalves, nh * 4), dtype=np.float16)
        for i in range(bpc):
            hh, ii = i // nh, i % nh
            gt40h[ii * 64:ii * 64 + m, hh, ii * 4:(ii + 1) * 4] = gtb[i].astype(np.float16)
        in_maps.append({
            "logits": np.ascontiguousarray(cls_logits[s]),
            "pred": np.ascontiguousarray(bbox_pred_cxcywh[s]),
            "db": np.ascontiguousarray(default_boxes_xyxy),
            "gtrep": gtrep,
            "gt40": gt40,
            "gt40h": gt40h,
            "ident": ident,
            "iota32": iota32,
            "iota81": iota81,
        })
    return in_maps


def finalize(outs, b=B, n=N):
    """outs: list of [1,32] per-core results -> (loss, loc_norm, conf_norm)."""
    acc = np.zeros(32, dtype=np.float64)
    for o in outs:
        acc += np.asarray(o).reshape(-1).astype(np.float64)
    loc_total, pos_lse, pcorr, negs, tp = acc[0], acc[1], acc[2], acc[3], acc[4]
    conf_total = (pos_lse - pcorr) + negs
    den = max(tp, 1.0)
    if tp > 0:
        loc_norm = loc_total / den
        conf_norm = conf_total / den
    else:
        loc_norm = 0.0
        conf_norm = conf_total / (b * n) if conf_total > 0 else 0.0
    return (np.float32(loc_norm + conf_norm), np.float32(loc_norm), np.float32(conf_norm))


def kernel(cls_logits, bbox_pred_cxcywh, gt_boxes, gt_labels, default_boxes_xyxy):
    nc = _get_nc()
    in_maps = host_prep(np.asarray(cls_logits), np.asarray(bbox_pred_cxcywh),
                        np.asarray(gt_boxes), np.asarray(gt_labels),
                        np.asarray(default_boxes_xyxy))
    res = run_bass_kernel_spmd(nc, in_maps, core_ids=list(range(NCORES)))
    outs = [res.results[i]["out"] for i in range(NCORES)]
    return finalize(outs)

